# revision 61
# baseline (speedup 1.0000x reference)
"""Trainium2 Bass kernel for the adaLN (DiT-style) dense transformer block.

Sharding: data-parallel over B — core b computes batch element b (B=8, 8 cores,
no collectives). Host-side prep folds the ENTIRE adaLN modulation into the
weights (mod = silu(c) @ ada_w.T + ada_b is per-batch weight algebra, not
activation math): per core we ship
  qkv_w' = qkv_w * W1[c],  qkv_b' = qkv_b + qkv_w @ B1   (h1 = xhat*W1 + B1)
  proj_w' = G1[c'] * proj_w,  gpb1 = G1*proj_b
  fc1_w' = fc1_w * W2[c],  fc1_b' = fc1_b + fc1_w @ B2
  fc2_w' = G2[c'] * fc2_w,  gpb2 = G2*fc2_b
so the device computes a plain pre-LN block: LN -> qkv -> attn -> proj(+res)
-> LN -> MLP(+res), with xhat-only LayerNorms.

All C/MLP-contraction matmuls (qkv, v, proj, fc1, fc2) run in fp8e4m3 with
MatmulPerfMode.DoubleRow (2 contraction rows per partition, 2x PE rate):
operands are chunk-pair-blocked [P, 2, F] access patterns. Folded weights are
pre-scaled by powers of 2 (qkv x8, fc1 x8, proj x32, fc2 x32) so the gated
folds (sigma ~0.005) sit in fp8's normal range; descales ride for free in the
exp scale (a further /64 from q*k), the gelu ACT scale (/8), and the fused
scalar_tensor_tensor residual adds (/256, /32).

Per-core dataflow (T=2048 tokens, C=512, H=8 heads, DH=64, MLP=2048),
~470-505us on HW (vs 640us bf16 baseline):
  - LN1 is a per-tile pipeline (stats -> Ln/Exp rstd -> xhat -> PE transpose
    -> one strided copy into the chunk-blocked [P, KC*T] fp8 h1T); Ln+Exp
    share one patched table set so per-tile pairs cost no reloads
  - attention is npair-OUTER (two full head passes over the q-column
    halves): everything the first half unlocks (proj, residual 1, LN2 for
    token tiles 0-7) rides the second pass's exp shadow, emitted at head
    BOTTOMS with 1-head (xhat) / 2-head (transpose) lags so nothing ever
    sits ahead of the next head's exps in an in-order queue; v matmuls and
    the next pair's qk blocks ride the h0/odd-head shadows; each unit's
    final o-matmuls + drain are deferred into the NEXT unit's tk0 slot
  - attention math per head (bf16): S.T tiles [tk, q] via lhsT=k.T, exp on
    ScalarE from PSUM (no max-subtraction — logits bounded), o via
    lhsT=[v|ones] so the denominator rides the same matmul; denominators
    gathered on partitions {0,32,64,96}, one reciprocal_approx_fast per
    4-head batch
  - proj/fc2 run "swapped" (lhsT=activations) so outputs land token-major
    and residuals are single fused DVE ops straight from PSUM; gate*bias
    rides a ones-row matmul; MLP n=0,1 (h2T shadow-complete) start right
    after the last exp while the second-half LN2 pipelines through DVE/PE
"""

import numpy as np
import ml_dtypes

import concourse.bass as bass
import concourse.bacc as bacc
import concourse.hw_specs as _hw_specs

# Route Exp and Ln to the one table set that holds BOTH
# (natural_log_exp_and_others). The default first-match assignment puts Exp in
# exp_and_others and Ln in natural_log, so every rstd = exp(-ln(v)/2) pair
# costs two 1.3us ACT table reloads. Blank those two sets (positions kept so
# act_func_set_ids stay aligned with act_info.json) and both functions
# first-match the combined set -> zero reloads.
if not getattr(_hw_specs.get_activation_tables, "_excl_exp_sets", False):
    _orig_get_tables = _hw_specs.get_activation_tables

    def _patched_get_tables(arch):
        t = _orig_get_tables(arch)
        for nm in ("exp_and_others", "natural_log"):
            if nm in t:
                t[nm] = set()
        return t

    _patched_get_tables._excl_exp_sets = True
    _hw_specs.get_activation_tables = _patched_get_tables
    bacc.get_activation_tables = _patched_get_tables
import concourse.tile as tile
import concourse.mybir as mybir
from concourse.bass_utils import run_bass_kernel_spmd
from concourse.masks import make_identity

F32 = mybir.dt.float32
BF16 = mybir.dt.bfloat16
FP8 = mybir.dt.float8e4
DR = mybir.MatmulPerfMode.DoubleRow
AF = mybir.ActivationFunctionType
ALU = mybir.AluOpType

B, T, C = 8, 2048, 512
H, DH, MLP = 8, 64, 4 * 512
P = 128
NT = T // P          # 16 token tiles
KC = C // P          # 4 feature chunks
NQ = T // 512        # 4 tq/tk column chunks of 512
EPS = 1e-5
SQ = 8.0             # qkv folded-weight (and bias) pre-scale
SP = 32.0            # proj folded-weight pre-scale
SM1 = 8.0            # fc1 folded-weight pre-scale
SM2 = 32.0           # fc2 folded-weight pre-scale
GELU_AF = AF.Gelu_apprx_tanh  # test.py sim swaps to Tanh (CoreSim lacks gelu)


def build_program():
    nc = bacc.Bacc("TRN2", target_bir_lowering=False, debug=False)

    # ---- DRAM I/O (all weights pre-folded + pre-scaled on host, per core) ----
    x_d = nc.dram_tensor("x", [NT, P, C], F32, kind="ExternalInput").ap()
    qkv_wt = nc.dram_tensor("qkv_wt", [KC, P, 3 * C], FP8, kind="ExternalInput").ap()
    proj_wt = nc.dram_tensor("proj_wt", [KC, P, C], FP8, kind="ExternalInput").ap()
    fc1_wt = nc.dram_tensor("fc1_wt", [KC, P, MLP], FP8, kind="ExternalInput").ap()
    fc2_wt = nc.dram_tensor("fc2_wt", [MLP // P, P, C], FP8, kind="ExternalInput").ap()
    qkv_b_qk = nc.dram_tensor("qkv_b_qk", [P, 8], F32, kind="ExternalInput").ap()
    fc1_b_c = nc.dram_tensor("fc1_b_c", [P, MLP // P], F32, kind="ExternalInput").ap()
    rows_d = {}
    for nm in ["vb_row", "gpb1", "gpb2"]:
        rows_d[nm] = nc.dram_tensor(nm, [1, C], BF16, kind="ExternalInput").ap()
    out_d = nc.dram_tensor("out", [NT, P, C], F32, kind="ExternalOutput").ap()
    # DRAM bounce buffer: partition-broadcast DMA needs a DRAM source
    rec_scr = nc.dram_tensor("rec_scr", [2 * H, 1024], BF16).ap()

    from contextlib import ExitStack
    with tile.TileContext(nc) as tc, ExitStack() as ctx:
        consts = ctx.enter_context(tc.tile_pool(name="consts", bufs=1))
        wbig = ctx.enter_context(tc.tile_pool(name="wbig", bufs=4))
        wsmall = ctx.enter_context(tc.tile_pool(name="wsmall", bufs=8))
        bigT = ctx.enter_context(tc.tile_pool(name="bigT", bufs=2))
        qk_pool = ctx.enter_context(tc.tile_pool(name="qk", bufs=8))
        vpool = ctx.enter_context(tc.tile_pool(name="vp", bufs=NT))
        work = ctx.enter_context(tc.tile_pool(name="work", bufs=2))
        projp = ctx.enter_context(tc.tile_pool(name="projp", bufs=2))
        psum = ctx.enter_context(tc.tile_pool(name="ps", bufs=2, space="PSUM"))

        # ---- persistent SBUF loads: x first (it gates LN1 stats), spread
        # across the 3 DMA-capable engine queues ----
        # qkv weights FIRST on the sync queue (786KB fp8, lands ~8us) so the
        # q/k half-blocks emitted between the LN1 halves aren't weight-gated
        qkv_sbp = []
        for u in range(2):
            w = wbig.tile([P, 2 * 3 * C], FP8, tag="wbig", name=f"qkvw{u}")
            for r in range(2):
                nc.sync.dma_start(w[:, r * 3 * C:(r + 1) * 3 * C], qkv_wt[2 * u + r])
            qkv_sbp.append(w)
        sx = []
        dmaq = [nc.scalar, nc.sync, nc.gpsimd]
        for i in range(NT):
            t = consts.tile([P, C], F32, name=f"x{i}")
            dmaq[i % 3].dma_start(t, x_d[i])
            sx.append(t)
        # remaining weight pair-tiles (chunk-pair-blocked for DoubleRow) on
        # sync so ACT/DVE/gpsimd queues stay free for early compute (a
        # dma_start trigger occupies its queue until ring space frees, so
        # weight loads on ACT would stall the LN chain).
        fc1_sbp = []
        for u in range(2):
            w = wbig.tile([P, 2 * MLP], FP8, tag="wbig", name=f"fc1w{u}")
            for r in range(2):
                nc.sync.dma_start(w[:, r * MLP:(r + 1) * MLP], fc1_wt[2 * u + r])
            fc1_sbp.append(w)
        proj_sbp = []
        for u in range(2):
            w = projp.tile([P, 2 * C], FP8, tag="projw", name=f"projw{u}")
            for r in range(2):
                nc.sync.dma_start(w[:, r * C:(r + 1) * C], proj_wt[2 * u + r])
            proj_sbp.append(w)
        fc2_sbp = []
        for u in range(MLP // P // 2):
            w = wsmall.tile([P, 2 * C], FP8, tag="wsmall", name=f"fc2w{u}")
            for r in range(2):
                nc.sync.dma_start(w[:, r * C:(r + 1) * C], fc2_wt[2 * u + r])
            fc2_sbp.append(w)

        def pair2(w):  # [P, 2*F] pair tile -> [P, 2, F] DoubleRow AP
            return w.rearrange("p (two f) -> p two f", two=2)

        ident = consts.tile([P, P], BF16, name="ident")
        make_identity(nc, ident)
        eps_t = consts.tile([P, 1], F32, name="eps_t")
        nc.gpsimd.memset(eps_t, EPS)
        qkvb_sb = consts.tile([P, 8], F32, name="qkvb_sb")
        nc.sync.dma_start(qkvb_sb, qkv_b_qk)
        fc1b_sb = consts.tile([P, MLP // P], F32, name="fc1b_sb")
        nc.sync.dma_start(fc1b_sb, fc1_b_c)
        row_sb = {}
        for nm in rows_d:
            t = consts.tile([1, C], BF16, name=nm + "_sb")
            nc.sync.dma_start(t, rows_d[nm])
            row_sb[nm] = t
        VBrow, GPB1row, GPB2row = (row_sb[n] for n in ("vb_row", "gpb1", "gpb2"))
        # softmax denominators collected on partitions {0,32,64,96} (the only
        # legal engine start-partitions) so ONE partition-parallel reciprocal
        # serves each 2-head batch. One tile reused across the 4 batches.
        den_all = consts.tile([P, 1024], F32, name="den_all")
        rec_f32 = consts.tile([P, 1024], F32, name="rec_f32")
        rec_all = consts.tile([P, 1024], BF16, name="rec_all")
        nc.gpsimd.memset(den_all, 1.0)
        ones_r = consts.tile([1, P], BF16, name="ones_r")
        nc.gpsimd.memset(ones_r, 1.0)

        def bcast(dst, src_row):
            src = bass.AP(tensor=src_row.tensor, offset=src_row.offset,
                          ap=[[0, dst.shape[0]]] + list(src_row.ap[1:]))
            nc.sync.dma_start(out=dst, in_=src)

        # ---- LN1: fully per-tile pipeline. Ln+Exp share one (patched)
        # table set, so per-tile rstd pairs cost no reloads and apply(i)
        # unblocks as soon as x[i] lands — a batched form would queue
        # apply(0) behind Ln(15) on the in-order ACT queue, gating
        # everything on the LAST x DMA. ----
        def ln_rstd(i, tag):
            st = work.tile([P, 6], F32, tag="st", bufs=2, name=f"st{tag}{i}")
            nc.vector.bn_stats(st, sx[i])
            mv = work.tile([P, 2], F32, tag="mv", bufs=NT, name=f"mv{tag}{i}")
            nc.vector.bn_aggr(mv, st)
            rstd = work.tile([P, 1], F32, tag="rstd", bufs=NT,
                             name=f"rstd{tag}{i}")
            nc.scalar.activation(rstd, mv[:, 1:2], AF.Ln, bias=eps_t)
            nc.scalar.activation(rstd, rstd, AF.Exp, scale=-0.5)
            negmr = work.tile([P, 1], F32, tag="negmr", bufs=NT,
                              name=f"negmr{tag}{i}")
            nc.vector.tensor_scalar(negmr, mv[:, 0:1], rstd, -1.0,
                                    op0=ALU.mult, op1=ALU.mult)
            return rstd, negmr

        # hT is ONE chunk-blocked fp8 tile [P, KC*T]: chunk k of token tile i
        # lives at columns [k*T + i*P, k*T + (i+1)*P). All 4 transposed
        # chunks of a token tile move with a single strided (casting) copy.
        def hT_dst(hT, i):
            return hT.rearrange("p (k t) -> p k t", k=KC)[:, :, i * P:(i + 1) * P]

        def hT_pair(hT, u, lo, hi):  # DoubleRow moving AP [P, 2, hi-lo]
            return hT.rearrange("p (k t) -> p k t", k=KC)[:, 2 * u:2 * u + 2, lo:hi]

        def ln_apply(xt, i, rstd, negmr, hT, stats_tag):
            # xhat only — the modulation affine lives in the folded weights.
            # Work alternates DVE/ACT so neither in-order queue serializes
            # the 16-tile chain.
            t1 = work.tile([P, C], BF16, tag="t1", bufs=8, name=f"t1{stats_tag}{i}")
            if i % 2 == 0:
                nc.vector.tensor_scalar(t1, xt, rstd, negmr, op0=ALU.mult,
                                        op1=ALU.add)
            else:
                nc.scalar.activation(t1, xt, AF.Identity, bias=negmr,
                                     scale=rstd)
            tp = psum.tile([P, C], BF16, tag="sg", bufs=2,
                           name=f"tp{stats_tag}_{i}")
            for j in range(KC):
                nc.tensor.transpose(tp[:, j * P:(j + 1) * P],
                                    t1[:, j * P:(j + 1) * P], ident)
            src = tp.rearrange("p (k t) -> p k t", k=KC)
            if i % 2 == 0:
                nc.vector.tensor_copy(hT_dst(hT, i), src)
            else:
                nc.scalar.copy(hT_dst(hT, i), src)

        h1T = bigT.tile([P, KC * T], FP8, tag="bigT", bufs=1, name="h1T")
        for i in range(8):
            rstd, negmr = ln_rstd(i, "a")
            ln_apply(sx[i], i, rstd, negmr, h1T, "a")

        # ---- qkv: q,k feature-major [8 x (P, T)]; v token-major interleaved ----
        # v: out token-major [t, c_v], scattered into [128, 8, 65] (| ones).
        # Only the first few v tiles run up front — the rest ride the exp
        # shadow of head 0 (emitted just-in-time inside the tk loop).
        vtok = [vpool.tile([P, H * 65], BF16, tag="vtok", name=f"vtok{i}")
                for i in range(NT)]

        def v_mms(i):
            ps = psum.tile([P, 1024], F32, tag="sg", name=f"vps{i}")
            for u in range(2):
                nc.tensor.matmul(ps[:, 0:C], hT_pair(h1T, u, i * P, (i + 1) * P),
                                 pair2(qkv_sbp[u])[:, :, 2 * C:3 * C],
                                 start=(u == 0), stop=False, perf_mode=DR)
            nc.tensor.matmul(ps[:, 0:C], ones_r[0:1, :], VBrow[0:1, :],
                             start=False, stop=True)
            src = ps[:, 0:C].rearrange("p (h d) -> p h d", h=H)
            dst3 = vtok[i].rearrange("p (h d) -> p h d", d=65)[:, :, 0:DH]
            nc.vector.tensor_copy(dst3, src)
            ones_col = vtok[i].rearrange("p (h d) -> p h d", d=65)[:, :, DH:65]
            nc.gpsimd.memset(ones_col, 1.0)

        qkT = [qk_pool.tile([P, T], BF16, tag="qk", name=f"qkT{m}") for m in range(8)]

        def qk_block(m, pps=(0, 1)):
            # pps selects 1024-column halves, so the first halves of blocks
            # 0/4 can be emitted as soon as LN1 has produced token tiles 0-7
            # (emission point controls the h1T tile-granular dependency).
            for pp in pps:
                prs = psum.tile([P, 1024], F32, tag="oaccp",
                                name=f"qkps{m}_{pp}")
                for u in range(2):
                    for n2 in range(2):
                        n = 2 * pp + n2
                        nc.tensor.matmul(prs[:, n2 * 512:(n2 + 1) * 512],
                                         pair2(qkv_sbp[u])[:, :, m * P:(m + 1) * P],
                                         hT_pair(h1T, u, n * 512, (n + 1) * 512),
                                         start=(u == 0), stop=(u == 1),
                                         perf_mode=DR)
                nc.vector.tensor_scalar(qkT[m][:, pp * 1024:(pp + 1) * 1024],
                                        prs, qkvb_sb[:, m:m + 1], None,
                                        op0=ALU.add)

        # ---- attention, npair-OUTER: the q-column halves are processed as
        # two full passes over the heads, so everything the first half
        # unlocks (proj, residual 1, LN2 for token tiles 0-7) runs in the
        # shadow of the second half's exp stream. qk blocks + v matmuls ride
        # the first pass's shadow too.
        # oT: per-(q-half, chunk-pair) fp8 tiles feeding the DoubleRow proj.
        # Separate tiles per q-half so the second half's normalize writes
        # can't create false tile-granular WARs against the first half's
        # proj reads. o is normalized (bf16 stage * 1/den) as it is cast.
        oTnp = [[bigT.tile([P, 2 * 1024], FP8, tag="oT", bufs=4,
                           name=f"oT{np_}_{v}") for v in range(2)]
                for np_ in range(2)]
        h2T = bigT.tile([P, KC * T], FP8, tag="bigT", bufs=1, name="h2T")
        stg_pool = ctx.enter_context(tc.tile_pool(name="stg", bufs=2))
        rc_pool = ctx.enter_context(tc.tile_pool(name="rc", bufs=2))

        def pairT(w):  # [P, 2*1024] q-half pair tile -> [P, 2, 1024] AP
            return w.rearrange("p (two f) -> p two f", two=2)

        def proj_tile(i):
            # proj_sbp columns are pre-scaled by G1*32 and the ones-row
            # matmul adds 256*G1*proj_b; one fused DVE op descales (/256)
            # and adds the residual straight from PSUM. ps rides the oaccp
            # tag so the scores/exp sg rotation never waits on it.
            np_, off = divmod(i * P, 1024)
            ps = psum.tile([P, 1024], F32, tag="oaccp", name=f"prps{i}")
            for u in range(2):
                nc.tensor.matmul(ps[:, 0:C],
                                 pairT(oTnp[np_][u])[:, :, off:off + P],
                                 pair2(proj_sbp[u]),
                                 start=(u == 0), stop=False, perf_mode=DR)
            nc.tensor.matmul(ps[:, 0:C], ones_r[0:1, :], GPB1row[0:1, :],
                             start=False, stop=True)
            nc.vector.scalar_tensor_tensor(sx[i], ps[:, 0:C], 1.0 / (SQ * SP),
                                           sx[i], op0=ALU.mult, op1=ALU.add)

        def ln2_stats(i):
            # DVE half of LN2: runs right after tile i's residual
            st = work.tile([P, 6], F32, tag="st", bufs=2, name=f"stb{i}")
            nc.vector.bn_stats(st, sx[i])
            mv = work.tile([P, 2], F32, tag="mv", bufs=NT, name=f"mvb{i}")
            nc.vector.bn_aggr(mv, st)
            return mv

        def ln2_xhat(i, mv, use_act):
            # ACT rstd (Ln/Exp share the exp table set, no reloads) + xhat.
            # In the exp shadow this is emitted one head LATE so the
            # in-order ACT queue never waits on fresh data.
            rstd = work.tile([P, 1], F32, tag="rstd", bufs=NT, name=f"rstdb{i}")
            nc.scalar.activation(rstd, mv[:, 1:2], AF.Ln, bias=eps_t)
            nc.scalar.activation(rstd, rstd, AF.Exp, scale=-0.5)
            negmr = work.tile([P, 1], F32, tag="negmr", bufs=NT,
                              name=f"negmrb{i}")
            nc.vector.tensor_scalar(negmr, mv[:, 0:1], rstd, -1.0,
                                    op0=ALU.mult, op1=ALU.mult)
            t1 = work.tile([P, C], BF16, tag="t1", bufs=8, name=f"t1b{i}")
            if use_act and i % 2 == 1:
                nc.scalar.activation(t1, sx[i], AF.Identity, bias=negmr,
                                     scale=rstd)
            else:
                nc.vector.tensor_scalar(t1, sx[i], rstd, negmr, op0=ALU.mult,
                                        op1=ALU.add)
            return t1

        def ln2_tr(i, t1, use_act):
            # transposes + h2T copy; in the shadow this runs two heads late
            # so the PE never waits on a fresh t1 ahead of the next scores.
            tp = psum.tile([P, C], BF16, tag="sg", bufs=2, name=f"tpb_{i}")
            for j in range(KC):
                nc.tensor.transpose(tp[:, j * P:(j + 1) * P],
                                    t1[:, j * P:(j + 1) * P], ident)
            src = tp.rearrange("p (k t) -> p k t", k=KC)
            if use_act and i % 2 == 1:
                nc.scalar.copy(hT_dst(h2T, i), src)
            else:
                nc.vector.tensor_copy(hT_dst(h2T, i), src)

        # v 0-2 and the first q-halves of blocks 0/4 emit between the LN1
        # halves (qkv weights were DMA'd first, so they aren't weight-gated)
        # — they run on PE while LN1 tiles 8-15 flow through DVE/ACT
        for i in range(3):
            v_mms(i)
        qk_block(0, (0,))
        qk_block(4, (0,))
        for i in range(8, NT):
            rstd, negmr = ln_rstd(i, "a")
            ln_apply(sx[i], i, rstd, negmr, h1T, "a")
        qk_block(4, (1,))
        qk_block(0, (1,))
        stgs = {}
        mvs2 = {}
        t1s = {}
        pend = None  # (oaccp, es_prev, h, npair) — drained in the NEXT unit

        def drain_unit(oaccp, es_prev, h, npair):
            # final o-matmuls + stage/den copies for a finished unit; called
            # from inside the next unit's tk0 slot so the next head's scores
            # (and hence its first exp) never queue behind this work.
            vlast = vtok[NT - 1][:, h * 65:h * 65 + 65]
            for n2 in range(2):
                nc.tensor.matmul(oaccp[0:65, n2 * 512:(n2 + 1) * 512], vlast,
                                 es_prev[:, n2 * 512:(n2 + 1) * 512],
                                 start=False, stop=True)
            u = h % 4  # unit within the 4-head den batch
            # unnormalized o to a bf16 stage (written at the partition base
            # its oT slice will use, so the normalize tensor_mul has all
            # operands on one partition range); den row into the partition-
            # stacked collector at partition 32*u
            pb = (h % 2) * DH
            stg = stg_pool.tile([P, 1024], BF16, tag="stg", bufs=4,
                                name=f"stg{h}_{npair}")
            nc.vector.tensor_copy(stg[pb:pb + DH, :], oaccp[0:DH, :])
            nc.vector.tensor_copy(den_all[32 * u:32 * u + 1, :],
                                  oaccp[DH:DH + 1, :])
            stgs[u] = stg
            if h % 4 == 3:
                nc.vector.reciprocal_approx_fast(rec_f32, den_all)
                with nc.allow_low_precision(reason="softmax recip in bf16"):
                    nc.vector.tensor_copy(rec_all, rec_f32)
                for u2 in range(4):
                    r = npair * 8 + (h - 3) + u2
                    nc.sync.dma_start(rec_scr[r:r + 1, :],
                                      rec_all[32 * u2:32 * u2 + 1, :])
                for u2 in range(4):
                    hh = (h - 3) + u2
                    r = npair * 8 + hh
                    # rbc/stage share the oT slice's base partition
                    # (SB+SB tensor_tensor verifier rule)
                    rbc = rc_pool.tile([P, 1024], BF16, tag="rbc", bufs=2,
                                       name=f"rb{r}")
                    pbase = (hh % 2) * DH
                    sub = rbc[pbase:pbase + DH, :]
                    bcast(sub, rec_scr[r:r + 1, :])
                    j = hh // 2  # feature chunk -> oTnp[npair][j//2] blk j%2
                    sl = oTnp[npair][j // 2][pbase:pbase + DH,
                                             (j % 2) * 1024:
                                             (j % 2) * 1024 + 1024]
                    nc.vector.tensor_mul(sl, stgs[u2][pbase:pbase + DH, :],
                                         sub)
                stgs.clear()

        for npair in range(2):
            for h in range(H):
                qh = qkT[h // 2][(h % 2) * DH:(h % 2) * DH + DH, :]
                kh = qkT[4 + h // 2][(h % 2) * DH:(h % 2) * DH + DH, :]
                oaccp = psum.tile([P, 1024], F32, tag="oaccp",
                                  name=f"oaccp{h}_{npair}")
                es_prev = None
                for tk in range(NT):
                    if npair == 0 and h == 0 and 1 <= tk and tk + 2 < NT:
                        v_mms(tk + 2)
                    # the next head-pair's q/k blocks ride the exp shadow of
                    # the current odd head, so no qk matmul ever sits between
                    # a head boundary and its first scores in the PE queue
                    if npair == 0 and h % 2 == 1 and h < 7:
                        if tk == 4:
                            qk_block((h + 1) // 2)
                        elif tk == 10:
                            qk_block(4 + (h + 1) // 2)
                    sg = psum.tile([P, 1024], F32, tag="sg",
                                   name=f"sg{h}_{npair}_{tk}")
                    for n2 in range(2):
                        n = 2 * npair + n2
                        nc.tensor.matmul(sg[:, n2 * 512:(n2 + 1) * 512],
                                         kh[:, tk * P:(tk + 1) * P],
                                         qh[:, n * 512:(n + 1) * 512],
                                         start=True, stop=True)
                    if tk == 0 and pend is not None:
                        drain_unit(*pend)
                    # o-matmuls run one tk behind so the in-order PE queue
                    # never waits on the exp of the current tk
                    if es_prev is not None:
                        vprev = vtok[tk - 1][:, h * 65:h * 65 + 65]
                        for n2 in range(2):
                            nc.tensor.matmul(
                                oaccp[0:65, n2 * 512:(n2 + 1) * 512], vprev,
                                es_prev[:, n2 * 512:(n2 + 1) * 512],
                                start=(tk - 1 == 0), stop=False)
                    es = work.tile([P, 1024], BF16, tag="es", bufs=3,
                                   name=f"es{h}_{npair}_{tk}")
                    # q and k both carry the x8 fold -> descale exp by /64
                    nc.scalar.activation(es, sg, AF.Exp, scale=0.125 / (SQ * SQ))
                    es_prev = es
                pend = (oaccp, es_prev, h, npair)
                if npair == 1:
                    # npair-0's proj/residual/LN2 (token tiles 0-7) ride this
                    # pass's exp shadow, emitted at head BOTTOMS so nothing
                    # sits ahead of the next head's exps in any in-order
                    # queue; xhat runs one head late, transposes two heads
                    # late, so no engine ever waits on fresh data.
                    proj_tile(h)
                    if h >= 1:
                        t1s[h - 1] = ln2_xhat(h - 1, mvs2[h - 1], use_act=False)
                    if h >= 2:
                        ln2_tr(h - 2, t1s[h - 2], use_act=False)
                    mvs2[h] = ln2_stats(h)
        drain_unit(*pend)

        # ---- post-attention tail + MLP ----
        # gelu descales fc1's x8 via its ACT scale and writes fp8 pair tiles
        # feeding the DoubleRow fc2.
        def mlp_n(n):
            fps = [psum.tile([P, 1024], F32, tag="oaccp", name=f"fps{n}_{sp}")
                   for sp in range(2)]

            def fc2_mms(u, g1p):
                for s in range(4):
                    nc.tensor.matmul(fps[s // 2][:, (s % 2) * 512:(s % 2) * 512 + 512],
                                     pair2(g1p)[:, :, s * P:(s + 1) * P],
                                     pair2(fc2_sbp[u]),
                                     start=(u == 0), stop=False, perf_mode=DR)

            g1_prev = None
            g1p = None
            for m in range(MLP // P):
                ps = psum.tile([P, 1024], F32, tag="sg", name=f"f1ps{n}_{m}")
                for u in range(2):
                    nc.tensor.matmul(ps[:, 0:C],
                                     pair2(fc1_sbp[u])[:, :, m * P:(m + 1) * P],
                                     hT_pair(h2T, u, n * 512, (n + 1) * 512),
                                     start=(u == 0), stop=(u == 1), perf_mode=DR)
                if m % 2 == 0:
                    g1p = work.tile([P, 2 * C], FP8, tag="g1", bufs=3,
                                    name=f"g1_{n}_{m}")
                nc.scalar.activation(g1p[:, (m % 2) * C:(m % 2) * C + C],
                                     ps[:, 0:C], GELU_AF,
                                     bias=fc1b_sb[:, m:m + 1], scale=1.0 / SM1)
                if m % 2 == 1:
                    if g1_prev is not None:
                        fc2_mms(m // 2 - 1, g1_prev)
                    g1_prev = g1p
            fc2_mms(MLP // P // 2 - 1, g1_prev)
            for s in range(4):
                nc.tensor.matmul(fps[s // 2][:, (s % 2) * 512:(s % 2) * 512 + 512],
                                 ones_r[0:1, :], GPB2row[0:1, :],
                                 start=False, stop=True)
            for s in range(4):
                i = n * 4 + s
                nc.vector.scalar_tensor_tensor(
                    sx[i], fps[s // 2][:, (s % 2) * 512:(s % 2) * 512 + 512],
                    1.0 / SM2, sx[i], op0=ALU.mult, op1=ALU.add)
                nc.sync.dma_start(out_d[i], sx[i])

        # finish the shadow LN2 pipeline (tiles 6,7); proj + stats for the
        # second q-half go FIRST on PE/DVE so their ACT rstds are ready the
        # moment gelu n=0 ends; MLP n=0,1 (h2T tiles shadow-complete) keeps
        # ACT busy while the second-half LN2 pipelines through DVE/PE.
        t1s[7] = ln2_xhat(7, mvs2[7], use_act=False)
        ln2_tr(6, t1s[6], use_act=False)
        ln2_tr(7, t1s[7], use_act=False)
        mlp_n(0)
        for i in range(8, NT):
            proj_tile(i)
            mvs2[i] = ln2_stats(i)
        mlp_n(1)
        for i in range(8, NT):
            t1s[i] = ln2_xhat(i, mvs2[i], use_act=False)
        for i in range(8, NT):
            ln2_tr(i, t1s[i], use_act=False)
        mlp_n(2)
        mlp_n(3)

    nc.compile()
    return nc


def make_in_maps(inputs):
    bf = ml_dtypes.bfloat16
    f8 = ml_dtypes.float8_e4m3
    f32 = np.float32
    f64 = np.float64
    x = np.asarray(inputs["x"], f32)
    c = np.asarray(inputs["c"], f64)
    qkv_w = np.asarray(inputs["qkv_w"], f64)
    qkv_b = np.asarray(inputs["qkv_b"], f64)
    proj_w = np.asarray(inputs["proj_w"], f64)
    proj_b = np.asarray(inputs["proj_b"], f64)
    ada_w = np.asarray(inputs["ada_w"], f64)
    ada_b = np.asarray(inputs["ada_b"], f64)
    fc1_w = np.asarray(inputs["fc1_w"], f64)
    fc1_b = np.asarray(inputs["fc1_b"], f64)
    fc2_w = np.asarray(inputs["fc2_w"], f64)
    fc2_b = np.asarray(inputs["fc2_b"], f64)
    ln = {k: np.asarray(inputs[k], f64) for k in
          ["ln1_w", "ln1_b", "ln2_w", "ln2_b"]}

    # adaLN modulation on host: mod = silu(c) @ ada_w.T + ada_b  [B, 6C]
    sil = c / (1.0 + np.exp(-c))
    mod = sil @ ada_w.T + ada_b
    sh1, sc1, g1m, sh2, sc2, g2m = np.split(mod, 6, axis=1)

    maps = []
    for b in range(B):
        # fold LN affine + modulation into the weights (per batch element):
        # h1 = xhat*W1 + B1, so  h1 @ Wl^T = xhat @ (Wl*W1)^T + B1@Wl^T
        W1 = ln["ln1_w"] * (1.0 + sc1[b])
        B1 = ln["ln1_b"] * (1.0 + sc1[b]) + sh1[b]
        W2 = ln["ln2_w"] * (1.0 + sc2[b])
        B2 = ln["ln2_b"] * (1.0 + sc2[b]) + sh2[b]
        qkv_wf = qkv_w * W1[None, :] * SQ
        qkv_bf = (qkv_b + qkv_w @ B1) * SQ
        fc1_wf = fc1_w * W2[None, :] * SM1
        fc1_bf = fc1_b + fc1_w @ B2          # unscaled: gelu scale descales
        proj_wf = g1m[b][:, None] * proj_w * SP
        fc2_wf = g2m[b][:, None] * fc2_w * SM2
        m = {
            "x": np.ascontiguousarray(x[b].reshape(NT, P, C)),
            "qkv_wt": np.ascontiguousarray(
                qkv_wf.T.reshape(KC, P, 3 * C)).astype(f8),
            "proj_wt": np.ascontiguousarray(
                proj_wf.T.reshape(KC, P, C)).astype(f8),
            "fc1_wt": np.ascontiguousarray(
                fc1_wf.T.reshape(KC, P, MLP)).astype(f8),
            "fc2_wt": np.ascontiguousarray(
                fc2_wf.T.reshape(MLP // P, P, C)).astype(f8),
            "qkv_b_qk": np.ascontiguousarray(
                qkv_bf[:2 * C].reshape(8, P).T).astype(f32),
            "fc1_b_c": np.ascontiguousarray(
                fc1_bf.reshape(MLP // P, P).T).astype(f32),
            "vb_row": qkv_bf[2 * C:].reshape(1, C).astype(bf),
            "gpb1": (g1m[b] * proj_b * SQ * SP).reshape(1, C).astype(bf),
            "gpb2": (g2m[b] * fc2_b * SM2).reshape(1, C).astype(bf),
        }
        maps.append(m)
    return maps


_CACHED_NC = None


def run(inputs, trace=False):
    global _CACHED_NC
    if _CACHED_NC is None:
        _CACHED_NC = build_program()
    maps = make_in_maps(inputs)
    res = run_bass_kernel_spmd(_CACHED_NC, maps, core_ids=list(range(B)),
                               trace=trace)
    out = np.stack([res.results[b]["out"].reshape(T, C) for b in range(B)])
    return out.astype(np.float32), res


def kernel(**inputs) -> np.ndarray:
    out, _ = run(inputs, trace=False)
    return out


# revision 62
# speedup vs baseline: 1.0094x; 1.0094x over previous
"""Trainium2 Bass kernel for the adaLN (DiT-style) dense transformer block.

Sharding: data-parallel over B — core b computes batch element b (B=8, 8 cores,
no collectives). Host-side prep folds the ENTIRE adaLN modulation into the
weights (mod = silu(c) @ ada_w.T + ada_b is per-batch weight algebra, not
activation math): per core we ship
  qkv_w' = qkv_w * W1[c],  qkv_b' = qkv_b + qkv_w @ B1   (h1 = xhat*W1 + B1)
  proj_w' = G1[c'] * proj_w,  gpb1 = G1*proj_b
  fc1_w' = fc1_w * W2[c],  fc1_b' = fc1_b + fc1_w @ B2
  fc2_w' = G2[c'] * fc2_w,  gpb2 = G2*fc2_b
so the device computes a plain pre-LN block: LN -> qkv -> attn -> proj(+res)
-> LN -> MLP(+res), with xhat-only LayerNorms.

All C/MLP-contraction matmuls (qkv, v, proj, fc1, fc2) run in fp8e4m3 with
MatmulPerfMode.DoubleRow (2 contraction rows per partition, 2x PE rate):
operands are chunk-pair-blocked [P, 2, F] access patterns. Folded weights are
pre-scaled by powers of 2 (qkv x8, fc1 x8, proj x32, fc2 x32) so the gated
folds (sigma ~0.005) sit in fp8's normal range; descales ride for free in the
exp scale (a further /64 from q*k), the gelu ACT scale (/8), and the fused
scalar_tensor_tensor residual adds (/256, /32).

Per-core dataflow (T=2048 tokens, C=512, H=8 heads, DH=64, MLP=2048),
~470-505us on HW (vs 640us bf16 baseline):
  - LN1 is a per-tile pipeline (stats -> Ln/Exp rstd -> xhat -> PE transpose
    -> one strided copy into the chunk-blocked [P, KC*T] fp8 h1T); Ln+Exp
    share one patched table set so per-tile pairs cost no reloads
  - attention is npair-OUTER (two full head passes over the q-column
    halves): everything the first half unlocks (proj, residual 1, LN2 for
    token tiles 0-7) rides the second pass's exp shadow, emitted at head
    BOTTOMS with 1-head (xhat) / 2-head (transpose) lags so nothing ever
    sits ahead of the next head's exps in an in-order queue; v matmuls and
    the next pair's qk blocks ride the h0/odd-head shadows; each unit's
    final o-matmuls + drain are deferred into the NEXT unit's tk0 slot
  - attention math per head (bf16): S.T tiles [tk, q] via lhsT=k.T, exp on
    ScalarE from PSUM (no max-subtraction — logits bounded), o via
    lhsT=[v|ones] so the denominator rides the same matmul; denominators
    gathered on partitions {0,32,64,96}, one reciprocal_approx_fast per
    4-head batch
  - proj/fc2 run "swapped" (lhsT=activations) so outputs land token-major
    and residuals are single fused DVE ops straight from PSUM; gate*bias
    rides a ones-row matmul; MLP n=0,1 (h2T shadow-complete) start right
    after the last exp while the second-half LN2 pipelines through DVE/PE
"""

import numpy as np
import ml_dtypes

import concourse.bass as bass
import concourse.bacc as bacc
import concourse.hw_specs as _hw_specs

# Route Exp and Ln to the one table set that holds BOTH
# (natural_log_exp_and_others). The default first-match assignment puts Exp in
# exp_and_others and Ln in natural_log, so every rstd = exp(-ln(v)/2) pair
# costs two 1.3us ACT table reloads. Blank those two sets (positions kept so
# act_func_set_ids stay aligned with act_info.json) and both functions
# first-match the combined set -> zero reloads.
if not getattr(_hw_specs.get_activation_tables, "_excl_exp_sets", False):
    _orig_get_tables = _hw_specs.get_activation_tables

    def _patched_get_tables(arch):
        t = _orig_get_tables(arch)
        for nm in ("exp_and_others", "natural_log"):
            if nm in t:
                t[nm] = set()
        return t

    _patched_get_tables._excl_exp_sets = True
    _hw_specs.get_activation_tables = _patched_get_tables
    bacc.get_activation_tables = _patched_get_tables
import concourse.tile as tile
import concourse.mybir as mybir
from concourse.bass_utils import run_bass_kernel_spmd
from concourse.masks import make_identity

F32 = mybir.dt.float32
BF16 = mybir.dt.bfloat16
FP8 = mybir.dt.float8e4
DR = mybir.MatmulPerfMode.DoubleRow
AF = mybir.ActivationFunctionType
ALU = mybir.AluOpType

B, T, C = 8, 2048, 512
H, DH, MLP = 8, 64, 4 * 512
P = 128
NT = T // P          # 16 token tiles
KC = C // P          # 4 feature chunks
NQ = T // 512        # 4 tq/tk column chunks of 512
EPS = 1e-5
SQ = 8.0             # qkv folded-weight (and bias) pre-scale
SP = 32.0            # proj folded-weight pre-scale
SM1 = 8.0            # fc1 folded-weight pre-scale
SM2 = 32.0           # fc2 folded-weight pre-scale
GELU_AF = AF.Gelu_apprx_tanh  # test.py sim swaps to Tanh (CoreSim lacks gelu)


def build_program():
    nc = bacc.Bacc("TRN2", target_bir_lowering=False, debug=False)

    # ---- DRAM I/O (all weights pre-folded + pre-scaled on host, per core) ----
    x_d = nc.dram_tensor("x", [NT, P, C], F32, kind="ExternalInput").ap()
    qkv_wt = nc.dram_tensor("qkv_wt", [KC, P, 3 * C], FP8, kind="ExternalInput").ap()
    proj_wt = nc.dram_tensor("proj_wt", [KC, P, C], FP8, kind="ExternalInput").ap()
    fc1_wt = nc.dram_tensor("fc1_wt", [KC, P, MLP], FP8, kind="ExternalInput").ap()
    fc2_wt = nc.dram_tensor("fc2_wt", [MLP // P, P, C], FP8, kind="ExternalInput").ap()
    qkv_b_qk = nc.dram_tensor("qkv_b_qk", [P, 8], F32, kind="ExternalInput").ap()
    fc1_b_c = nc.dram_tensor("fc1_b_c", [P, MLP // P], F32, kind="ExternalInput").ap()
    rows_d = {}
    for nm in ["vb_row", "gpb1", "gpb2"]:
        rows_d[nm] = nc.dram_tensor(nm, [1, C], BF16, kind="ExternalInput").ap()
    out_d = nc.dram_tensor("out", [NT, P, C], F32, kind="ExternalOutput").ap()
    # DRAM bounce buffer: partition-broadcast DMA needs a DRAM source
    rec_scr = nc.dram_tensor("rec_scr", [2 * H, 1024], BF16).ap()

    from contextlib import ExitStack
    with tile.TileContext(nc) as tc, ExitStack() as ctx:
        consts = ctx.enter_context(tc.tile_pool(name="consts", bufs=1))
        wbig = ctx.enter_context(tc.tile_pool(name="wbig", bufs=4))
        wsmall = ctx.enter_context(tc.tile_pool(name="wsmall", bufs=8))
        bigT = ctx.enter_context(tc.tile_pool(name="bigT", bufs=2))
        qk_pool = ctx.enter_context(tc.tile_pool(name="qk", bufs=8))
        vpool = ctx.enter_context(tc.tile_pool(name="vp", bufs=NT))
        work = ctx.enter_context(tc.tile_pool(name="work", bufs=2))
        projp = ctx.enter_context(tc.tile_pool(name="projp", bufs=2))
        psum = ctx.enter_context(tc.tile_pool(name="ps", bufs=2, space="PSUM"))

        # ---- persistent SBUF loads: x first (it gates LN1 stats), spread
        # across the 3 DMA-capable engine queues ----
        # qkv weights FIRST on the sync queue (786KB fp8, lands ~8us) so the
        # q/k half-blocks emitted between the LN1 halves aren't weight-gated
        qkv_sbp = []
        for u in range(2):
            w = wbig.tile([P, 2 * 3 * C], FP8, tag="wbig", name=f"qkvw{u}")
            for r in range(2):
                nc.sync.dma_start(w[:, r * 3 * C:(r + 1) * 3 * C], qkv_wt[2 * u + r])
            qkv_sbp.append(w)
        # dummy Ln at t~0: forces the (shared Ln+Exp) act-table load to
        # overlap the x DMA wait instead of sitting on the LN1 critical path
        warm = consts.tile([P, 1], F32, name="warm")
        nc.gpsimd.memset(warm, 1.0)
        nc.scalar.activation(warm, warm, AF.Ln)
        sx = []
        dmaq = [nc.scalar, nc.sync, nc.gpsimd]
        for i in range(NT):
            t = consts.tile([P, C], F32, name=f"x{i}")
            dmaq[i % 3].dma_start(t, x_d[i])
            sx.append(t)
        # remaining weight pair-tiles (chunk-pair-blocked for DoubleRow) on
        # sync so ACT/DVE/gpsimd queues stay free for early compute (a
        # dma_start trigger occupies its queue until ring space frees, so
        # weight loads on ACT would stall the LN chain).
        fc1_sbp = []
        for u in range(2):
            w = wbig.tile([P, 2 * MLP], FP8, tag="wbig", name=f"fc1w{u}")
            for r in range(2):
                nc.sync.dma_start(w[:, r * MLP:(r + 1) * MLP], fc1_wt[2 * u + r])
            fc1_sbp.append(w)
        proj_sbp = []
        for u in range(2):
            w = projp.tile([P, 2 * C], FP8, tag="projw", name=f"projw{u}")
            for r in range(2):
                nc.sync.dma_start(w[:, r * C:(r + 1) * C], proj_wt[2 * u + r])
            proj_sbp.append(w)
        fc2_sbp = []
        for u in range(MLP // P // 2):
            w = wsmall.tile([P, 2 * C], FP8, tag="wsmall", name=f"fc2w{u}")
            for r in range(2):
                nc.sync.dma_start(w[:, r * C:(r + 1) * C], fc2_wt[2 * u + r])
            fc2_sbp.append(w)

        def pair2(w):  # [P, 2*F] pair tile -> [P, 2, F] DoubleRow AP
            return w.rearrange("p (two f) -> p two f", two=2)

        ident = consts.tile([P, P], BF16, name="ident")
        make_identity(nc, ident)
        eps_t = consts.tile([P, 1], F32, name="eps_t")
        nc.gpsimd.memset(eps_t, EPS)
        qkvb_sb = consts.tile([P, 8], F32, name="qkvb_sb")
        nc.sync.dma_start(qkvb_sb, qkv_b_qk)
        fc1b_sb = consts.tile([P, MLP // P], F32, name="fc1b_sb")
        nc.sync.dma_start(fc1b_sb, fc1_b_c)
        row_sb = {}
        for nm in rows_d:
            t = consts.tile([1, C], BF16, name=nm + "_sb")
            nc.sync.dma_start(t, rows_d[nm])
            row_sb[nm] = t
        VBrow, GPB1row, GPB2row = (row_sb[n] for n in ("vb_row", "gpb1", "gpb2"))
        # softmax denominators collected on partitions {0,32,64,96} (the only
        # legal engine start-partitions) so ONE partition-parallel reciprocal
        # serves each 2-head batch. One tile reused across the 4 batches.
        den_all = consts.tile([P, 1024], F32, name="den_all")
        rec_f32 = consts.tile([P, 1024], F32, name="rec_f32")
        rec_all = consts.tile([P, 1024], BF16, name="rec_all")
        nc.gpsimd.memset(den_all, 1.0)
        ones_r = consts.tile([1, P], BF16, name="ones_r")
        nc.gpsimd.memset(ones_r, 1.0)

        def bcast(dst, src_row):
            src = bass.AP(tensor=src_row.tensor, offset=src_row.offset,
                          ap=[[0, dst.shape[0]]] + list(src_row.ap[1:]))
            nc.sync.dma_start(out=dst, in_=src)

        # ---- LN1: fully per-tile pipeline. Ln+Exp share one (patched)
        # table set, so per-tile rstd pairs cost no reloads and apply(i)
        # unblocks as soon as x[i] lands — a batched form would queue
        # apply(0) behind Ln(15) on the in-order ACT queue, gating
        # everything on the LAST x DMA. ----
        def ln_rstd(i, tag):
            st = work.tile([P, 6], F32, tag="st", bufs=2, name=f"st{tag}{i}")
            nc.vector.bn_stats(st, sx[i])
            mv = work.tile([P, 2], F32, tag="mv", bufs=NT, name=f"mv{tag}{i}")
            nc.vector.bn_aggr(mv, st)
            rstd = work.tile([P, 1], F32, tag="rstd", bufs=NT,
                             name=f"rstd{tag}{i}")
            nc.scalar.activation(rstd, mv[:, 1:2], AF.Ln, bias=eps_t)
            nc.scalar.activation(rstd, rstd, AF.Exp, scale=-0.5)
            negmr = work.tile([P, 1], F32, tag="negmr", bufs=NT,
                              name=f"negmr{tag}{i}")
            nc.vector.tensor_scalar(negmr, mv[:, 0:1], rstd, -1.0,
                                    op0=ALU.mult, op1=ALU.mult)
            return rstd, negmr

        # hT is ONE chunk-blocked fp8 tile [P, KC*T]: chunk k of token tile i
        # lives at columns [k*T + i*P, k*T + (i+1)*P). All 4 transposed
        # chunks of a token tile move with a single strided (casting) copy.
        def hT_dst(hT, i):
            return hT.rearrange("p (k t) -> p k t", k=KC)[:, :, i * P:(i + 1) * P]

        def hT_pair(hT, u, lo, hi):  # DoubleRow moving AP [P, 2, hi-lo]
            return hT.rearrange("p (k t) -> p k t", k=KC)[:, 2 * u:2 * u + 2, lo:hi]

        def ln_apply(xt, i, rstd, negmr, hT, stats_tag):
            # xhat only — the modulation affine lives in the folded weights.
            # Work alternates DVE/ACT so neither in-order queue serializes
            # the 16-tile chain.
            t1 = work.tile([P, C], BF16, tag="t1", bufs=8, name=f"t1{stats_tag}{i}")
            if i % 2 == 0:
                nc.vector.tensor_scalar(t1, xt, rstd, negmr, op0=ALU.mult,
                                        op1=ALU.add)
            else:
                nc.scalar.activation(t1, xt, AF.Identity, bias=negmr,
                                     scale=rstd)
            tp = psum.tile([P, C], BF16, tag="sg", bufs=2,
                           name=f"tp{stats_tag}_{i}")
            for j in range(KC):
                nc.tensor.transpose(tp[:, j * P:(j + 1) * P],
                                    t1[:, j * P:(j + 1) * P], ident)
            src = tp.rearrange("p (k t) -> p k t", k=KC)
            if i % 2 == 0:
                nc.vector.tensor_copy(hT_dst(hT, i), src)
            else:
                nc.scalar.copy(hT_dst(hT, i), src)

        h1T = bigT.tile([P, KC * T], FP8, tag="bigT", bufs=1, name="h1T")
        for i in range(8):
            rstd, negmr = ln_rstd(i, "a")
            ln_apply(sx[i], i, rstd, negmr, h1T, "a")

        # ---- qkv: q,k feature-major [8 x (P, T)]; v token-major interleaved ----
        # v: out token-major [t, c_v], scattered into [128, 8, 65] (| ones).
        # Only the first few v tiles run up front — the rest ride the exp
        # shadow of head 0 (emitted just-in-time inside the tk loop).
        vtok = [vpool.tile([P, H * 65], BF16, tag="vtok", name=f"vtok{i}")
                for i in range(NT)]

        def v_mms(i):
            ps = psum.tile([P, 1024], F32, tag="sg", name=f"vps{i}")
            for u in range(2):
                nc.tensor.matmul(ps[:, 0:C], hT_pair(h1T, u, i * P, (i + 1) * P),
                                 pair2(qkv_sbp[u])[:, :, 2 * C:3 * C],
                                 start=(u == 0), stop=False, perf_mode=DR)
            nc.tensor.matmul(ps[:, 0:C], ones_r[0:1, :], VBrow[0:1, :],
                             start=False, stop=True)
            src = ps[:, 0:C].rearrange("p (h d) -> p h d", h=H)
            dst3 = vtok[i].rearrange("p (h d) -> p h d", d=65)[:, :, 0:DH]
            nc.vector.tensor_copy(dst3, src)
            ones_col = vtok[i].rearrange("p (h d) -> p h d", d=65)[:, :, DH:65]
            nc.gpsimd.memset(ones_col, 1.0)

        qkT = [qk_pool.tile([P, T], BF16, tag="qk", name=f"qkT{m}") for m in range(8)]

        def qk_block(m, pps=(0, 1)):
            # pps selects 1024-column halves, so the first halves of blocks
            # 0/4 can be emitted as soon as LN1 has produced token tiles 0-7
            # (emission point controls the h1T tile-granular dependency).
            for pp in pps:
                prs = psum.tile([P, 1024], F32, tag="oaccp",
                                name=f"qkps{m}_{pp}")
                for u in range(2):
                    for n2 in range(2):
                        n = 2 * pp + n2
                        nc.tensor.matmul(prs[:, n2 * 512:(n2 + 1) * 512],
                                         pair2(qkv_sbp[u])[:, :, m * P:(m + 1) * P],
                                         hT_pair(h1T, u, n * 512, (n + 1) * 512),
                                         start=(u == 0), stop=(u == 1),
                                         perf_mode=DR)
                nc.vector.tensor_scalar(qkT[m][:, pp * 1024:(pp + 1) * 1024],
                                        prs, qkvb_sb[:, m:m + 1], None,
                                        op0=ALU.add)

        # ---- attention, npair-OUTER: the q-column halves are processed as
        # two full passes over the heads, so everything the first half
        # unlocks (proj, residual 1, LN2 for token tiles 0-7) runs in the
        # shadow of the second half's exp stream. qk blocks + v matmuls ride
        # the first pass's shadow too.
        # oT: per-(q-half, chunk-pair) fp8 tiles feeding the DoubleRow proj.
        # Separate tiles per q-half so the second half's normalize writes
        # can't create false tile-granular WARs against the first half's
        # proj reads. o is normalized (bf16 stage * 1/den) as it is cast.
        oTnp = [[bigT.tile([P, 2 * 1024], FP8, tag="oT", bufs=4,
                           name=f"oT{np_}_{v}") for v in range(2)]
                for np_ in range(2)]
        h2T = bigT.tile([P, KC * T], FP8, tag="bigT", bufs=1, name="h2T")
        stg_pool = ctx.enter_context(tc.tile_pool(name="stg", bufs=2))
        rc_pool = ctx.enter_context(tc.tile_pool(name="rc", bufs=2))

        def pairT(w):  # [P, 2*1024] q-half pair tile -> [P, 2, 1024] AP
            return w.rearrange("p (two f) -> p two f", two=2)

        def proj_tile(i):
            # proj_sbp columns are pre-scaled by G1*32 and the ones-row
            # matmul adds 256*G1*proj_b; one fused DVE op descales (/256)
            # and adds the residual straight from PSUM. ps rides the oaccp
            # tag so the scores/exp sg rotation never waits on it.
            np_, off = divmod(i * P, 1024)
            ps = psum.tile([P, 1024], F32, tag="oaccp", name=f"prps{i}")
            for u in range(2):
                nc.tensor.matmul(ps[:, 0:C],
                                 pairT(oTnp[np_][u])[:, :, off:off + P],
                                 pair2(proj_sbp[u]),
                                 start=(u == 0), stop=False, perf_mode=DR)
            nc.tensor.matmul(ps[:, 0:C], ones_r[0:1, :], GPB1row[0:1, :],
                             start=False, stop=True)
            nc.vector.scalar_tensor_tensor(sx[i], ps[:, 0:C], 1.0 / (SQ * SP),
                                           sx[i], op0=ALU.mult, op1=ALU.add)

        def ln2_stats(i):
            # DVE half of LN2: runs right after tile i's residual
            st = work.tile([P, 6], F32, tag="st", bufs=2, name=f"stb{i}")
            nc.vector.bn_stats(st, sx[i])
            mv = work.tile([P, 2], F32, tag="mv", bufs=NT, name=f"mvb{i}")
            nc.vector.bn_aggr(mv, st)
            return mv

        def ln2_xhat(i, mv, use_act):
            # ACT rstd (Ln/Exp share the exp table set, no reloads) + xhat.
            # In the exp shadow this is emitted one head LATE so the
            # in-order ACT queue never waits on fresh data.
            rstd = work.tile([P, 1], F32, tag="rstd", bufs=NT, name=f"rstdb{i}")
            nc.scalar.activation(rstd, mv[:, 1:2], AF.Ln, bias=eps_t)
            nc.scalar.activation(rstd, rstd, AF.Exp, scale=-0.5)
            negmr = work.tile([P, 1], F32, tag="negmr", bufs=NT,
                              name=f"negmrb{i}")
            nc.vector.tensor_scalar(negmr, mv[:, 0:1], rstd, -1.0,
                                    op0=ALU.mult, op1=ALU.mult)
            t1 = work.tile([P, C], BF16, tag="t1", bufs=8, name=f"t1b{i}")
            if use_act and i % 2 == 1:
                nc.scalar.activation(t1, sx[i], AF.Identity, bias=negmr,
                                     scale=rstd)
            else:
                nc.vector.tensor_scalar(t1, sx[i], rstd, negmr, op0=ALU.mult,
                                        op1=ALU.add)
            return t1

        def ln2_tr(i, t1, use_act):
            # transposes + h2T copy; in the shadow this runs two heads late
            # so the PE never waits on a fresh t1 ahead of the next scores.
            tp = psum.tile([P, C], BF16, tag="sg", bufs=2, name=f"tpb_{i}")
            for j in range(KC):
                nc.tensor.transpose(tp[:, j * P:(j + 1) * P],
                                    t1[:, j * P:(j + 1) * P], ident)
            src = tp.rearrange("p (k t) -> p k t", k=KC)
            if use_act and i % 2 == 1:
                nc.scalar.copy(hT_dst(h2T, i), src)
            else:
                nc.vector.tensor_copy(hT_dst(h2T, i), src)

        # v 0-2 and the first q-halves of blocks 0/4 emit between the LN1
        # halves (qkv weights were DMA'd first, so they aren't weight-gated)
        # — they run on PE while LN1 tiles 8-15 flow through DVE/ACT
        for i in range(3):
            v_mms(i)
        qk_block(0, (0,))
        qk_block(4, (0,))
        for i in range(8, NT):
            rstd, negmr = ln_rstd(i, "a")
            ln_apply(sx[i], i, rstd, negmr, h1T, "a")
        qk_block(4, (1,))
        qk_block(0, (1,))
        stgs = {}
        mvs2 = {}
        t1s = {}
        pend = None  # (oaccp, es_prev, h, npair) — drained in the NEXT unit

        def drain_unit(oaccp, es_prev, h, npair):
            # final o-matmuls + stage/den copies for a finished unit; called
            # from inside the next unit's tk0 slot so the next head's scores
            # (and hence its first exp) never queue behind this work.
            vlast = vtok[NT - 1][:, h * 65:h * 65 + 65]
            for n2 in range(2):
                nc.tensor.matmul(oaccp[0:65, n2 * 512:(n2 + 1) * 512], vlast,
                                 es_prev[:, n2 * 512:(n2 + 1) * 512],
                                 start=False, stop=True)
            u = h % 4  # unit within the 4-head den batch
            # unnormalized o to a bf16 stage (written at the partition base
            # its oT slice will use, so the normalize tensor_mul has all
            # operands on one partition range); den row into the partition-
            # stacked collector at partition 32*u
            pb = (h % 2) * DH
            stg = stg_pool.tile([P, 1024], BF16, tag="stg", bufs=4,
                                name=f"stg{h}_{npair}")
            nc.vector.tensor_copy(stg[pb:pb + DH, :], oaccp[0:DH, :])
            nc.vector.tensor_copy(den_all[32 * u:32 * u + 1, :],
                                  oaccp[DH:DH + 1, :])
            stgs[u] = stg
            if h % 4 == 3:
                nc.vector.reciprocal_approx_fast(rec_f32, den_all)
                with nc.allow_low_precision(reason="softmax recip in bf16"):
                    nc.vector.tensor_copy(rec_all, rec_f32)
                for u2 in range(4):
                    r = npair * 8 + (h - 3) + u2
                    nc.sync.dma_start(rec_scr[r:r + 1, :],
                                      rec_all[32 * u2:32 * u2 + 1, :])
                for u2 in range(4):
                    hh = (h - 3) + u2
                    r = npair * 8 + hh
                    # rbc/stage share the oT slice's base partition
                    # (SB+SB tensor_tensor verifier rule)
                    rbc = rc_pool.tile([P, 1024], BF16, tag="rbc", bufs=2,
                                       name=f"rb{r}")
                    pbase = (hh % 2) * DH
                    sub = rbc[pbase:pbase + DH, :]
                    bcast(sub, rec_scr[r:r + 1, :])
                    j = hh // 2  # feature chunk -> oTnp[npair][j//2] blk j%2
                    sl = oTnp[npair][j // 2][pbase:pbase + DH,
                                             (j % 2) * 1024:
                                             (j % 2) * 1024 + 1024]
                    nc.vector.tensor_mul(sl, stgs[u2][pbase:pbase + DH, :],
                                         sub)
                stgs.clear()

        for npair in range(2):
            for h in range(H):
                qh = qkT[h // 2][(h % 2) * DH:(h % 2) * DH + DH, :]
                kh = qkT[4 + h // 2][(h % 2) * DH:(h % 2) * DH + DH, :]
                oaccp = psum.tile([P, 1024], F32, tag="oaccp",
                                  name=f"oaccp{h}_{npair}")
                es_prev = None
                for tk in range(NT):
                    if npair == 0 and h == 0 and 1 <= tk and tk + 2 < NT:
                        v_mms(tk + 2)
                    # the next head-pair's q/k blocks ride the exp shadow of
                    # the current odd head, so no qk matmul ever sits between
                    # a head boundary and its first scores in the PE queue
                    if npair == 0 and h % 2 == 1 and h < 7:
                        if tk == 4:
                            qk_block((h + 1) // 2)
                        elif tk == 10:
                            qk_block(4 + (h + 1) // 2)
                    sg = psum.tile([P, 1024], F32, tag="sg",
                                   name=f"sg{h}_{npair}_{tk}")
                    for n2 in range(2):
                        n = 2 * npair + n2
                        nc.tensor.matmul(sg[:, n2 * 512:(n2 + 1) * 512],
                                         kh[:, tk * P:(tk + 1) * P],
                                         qh[:, n * 512:(n + 1) * 512],
                                         start=True, stop=True)
                    if tk == 0 and pend is not None:
                        drain_unit(*pend)
                    # o-matmuls run one tk behind so the in-order PE queue
                    # never waits on the exp of the current tk
                    if es_prev is not None:
                        vprev = vtok[tk - 1][:, h * 65:h * 65 + 65]
                        for n2 in range(2):
                            nc.tensor.matmul(
                                oaccp[0:65, n2 * 512:(n2 + 1) * 512], vprev,
                                es_prev[:, n2 * 512:(n2 + 1) * 512],
                                start=(tk - 1 == 0), stop=False)
                    es = work.tile([P, 1024], BF16, tag="es", bufs=3,
                                   name=f"es{h}_{npair}_{tk}")
                    # q and k both carry the x8 fold -> descale exp by /64
                    nc.scalar.activation(es, sg, AF.Exp, scale=0.125 / (SQ * SQ))
                    es_prev = es
                pend = (oaccp, es_prev, h, npair)
                if npair == 1:
                    # npair-0's proj/residual/LN2 (token tiles 0-7) ride this
                    # pass's exp shadow, emitted at head BOTTOMS so nothing
                    # sits ahead of the next head's exps in any in-order
                    # queue; xhat runs one head late, transposes two heads
                    # late, so no engine ever waits on fresh data.
                    proj_tile(h)
                    if h >= 1:
                        t1s[h - 1] = ln2_xhat(h - 1, mvs2[h - 1], use_act=False)
                    if h >= 2:
                        ln2_tr(h - 2, t1s[h - 2], use_act=False)
                    mvs2[h] = ln2_stats(h)
        drain_unit(*pend)

        # ---- post-attention tail + MLP ----
        # gelu descales fc1's x8 via its ACT scale and writes fp8 pair tiles
        # feeding the DoubleRow fc2.
        def mlp_n(n):
            fps = [psum.tile([P, 1024], F32, tag="oaccp", name=f"fps{n}_{sp}")
                   for sp in range(2)]

            def fc2_mms(u, g1p):
                for s in range(4):
                    nc.tensor.matmul(fps[s // 2][:, (s % 2) * 512:(s % 2) * 512 + 512],
                                     pair2(g1p)[:, :, s * P:(s + 1) * P],
                                     pair2(fc2_sbp[u]),
                                     start=(u == 0), stop=False, perf_mode=DR)

            g1_prev = None
            g1p = None
            for m in range(MLP // P):
                ps = psum.tile([P, 1024], F32, tag="sg", name=f"f1ps{n}_{m}")
                for u in range(2):
                    nc.tensor.matmul(ps[:, 0:C],
                                     pair2(fc1_sbp[u])[:, :, m * P:(m + 1) * P],
                                     hT_pair(h2T, u, n * 512, (n + 1) * 512),
                                     start=(u == 0), stop=(u == 1), perf_mode=DR)
                if m % 2 == 0:
                    g1p = work.tile([P, 2 * C], FP8, tag="g1", bufs=3,
                                    name=f"g1_{n}_{m}")
                nc.scalar.activation(g1p[:, (m % 2) * C:(m % 2) * C + C],
                                     ps[:, 0:C], GELU_AF,
                                     bias=fc1b_sb[:, m:m + 1], scale=1.0 / SM1)
                if m % 2 == 1:
                    if g1_prev is not None:
                        fc2_mms(m // 2 - 1, g1_prev)
                    g1_prev = g1p
            fc2_mms(MLP // P // 2 - 1, g1_prev)
            for s in range(4):
                nc.tensor.matmul(fps[s // 2][:, (s % 2) * 512:(s % 2) * 512 + 512],
                                 ones_r[0:1, :], GPB2row[0:1, :],
                                 start=False, stop=True)
            for s in range(4):
                i = n * 4 + s
                nc.vector.scalar_tensor_tensor(
                    sx[i], fps[s // 2][:, (s % 2) * 512:(s % 2) * 512 + 512],
                    1.0 / SM2, sx[i], op0=ALU.mult, op1=ALU.add)
                nc.sync.dma_start(out_d[i], sx[i])

        # finish the shadow LN2 pipeline (tiles 6,7); proj + stats for the
        # second q-half go FIRST on PE/DVE so their ACT rstds are ready the
        # moment gelu n=0 ends; MLP n=0,1 (h2T tiles shadow-complete) keeps
        # ACT busy while the second-half LN2 pipelines through DVE/PE.
        t1s[7] = ln2_xhat(7, mvs2[7], use_act=False)
        ln2_tr(6, t1s[6], use_act=False)
        ln2_tr(7, t1s[7], use_act=False)
        mlp_n(0)
        for i in range(8, NT):
            proj_tile(i)
            mvs2[i] = ln2_stats(i)
        mlp_n(1)
        for i in range(8, NT):
            t1s[i] = ln2_xhat(i, mvs2[i], use_act=False)
        for i in range(8, NT):
            ln2_tr(i, t1s[i], use_act=False)
        mlp_n(2)
        mlp_n(3)

    nc.compile()
    return nc


def make_in_maps(inputs):
    bf = ml_dtypes.bfloat16
    f8 = ml_dtypes.float8_e4m3
    f32 = np.float32
    f64 = np.float64
    x = np.asarray(inputs["x"], f32)
    c = np.asarray(inputs["c"], f64)
    qkv_w = np.asarray(inputs["qkv_w"], f64)
    qkv_b = np.asarray(inputs["qkv_b"], f64)
    proj_w = np.asarray(inputs["proj_w"], f64)
    proj_b = np.asarray(inputs["proj_b"], f64)
    ada_w = np.asarray(inputs["ada_w"], f64)
    ada_b = np.asarray(inputs["ada_b"], f64)
    fc1_w = np.asarray(inputs["fc1_w"], f64)
    fc1_b = np.asarray(inputs["fc1_b"], f64)
    fc2_w = np.asarray(inputs["fc2_w"], f64)
    fc2_b = np.asarray(inputs["fc2_b"], f64)
    ln = {k: np.asarray(inputs[k], f64) for k in
          ["ln1_w", "ln1_b", "ln2_w", "ln2_b"]}

    # adaLN modulation on host: mod = silu(c) @ ada_w.T + ada_b  [B, 6C]
    sil = c / (1.0 + np.exp(-c))
    mod = sil @ ada_w.T + ada_b
    sh1, sc1, g1m, sh2, sc2, g2m = np.split(mod, 6, axis=1)

    maps = []
    for b in range(B):
        # fold LN affine + modulation into the weights (per batch element):
        # h1 = xhat*W1 + B1, so  h1 @ Wl^T = xhat @ (Wl*W1)^T + B1@Wl^T
        W1 = ln["ln1_w"] * (1.0 + sc1[b])
        B1 = ln["ln1_b"] * (1.0 + sc1[b]) + sh1[b]
        W2 = ln["ln2_w"] * (1.0 + sc2[b])
        B2 = ln["ln2_b"] * (1.0 + sc2[b]) + sh2[b]
        qkv_wf = qkv_w * W1[None, :] * SQ
        qkv_bf = (qkv_b + qkv_w @ B1) * SQ
        fc1_wf = fc1_w * W2[None, :] * SM1
        fc1_bf = fc1_b + fc1_w @ B2          # unscaled: gelu scale descales
        proj_wf = g1m[b][:, None] * proj_w * SP
        fc2_wf = g2m[b][:, None] * fc2_w * SM2
        m = {
            "x": np.ascontiguousarray(x[b].reshape(NT, P, C)),
            "qkv_wt": np.ascontiguousarray(
                qkv_wf.T.reshape(KC, P, 3 * C)).astype(f8),
            "proj_wt": np.ascontiguousarray(
                proj_wf.T.reshape(KC, P, C)).astype(f8),
            "fc1_wt": np.ascontiguousarray(
                fc1_wf.T.reshape(KC, P, MLP)).astype(f8),
            "fc2_wt": np.ascontiguousarray(
                fc2_wf.T.reshape(MLP // P, P, C)).astype(f8),
            "qkv_b_qk": np.ascontiguousarray(
                qkv_bf[:2 * C].reshape(8, P).T).astype(f32),
            "fc1_b_c": np.ascontiguousarray(
                fc1_bf.reshape(MLP // P, P).T).astype(f32),
            "vb_row": qkv_bf[2 * C:].reshape(1, C).astype(bf),
            "gpb1": (g1m[b] * proj_b * SQ * SP).reshape(1, C).astype(bf),
            "gpb2": (g2m[b] * fc2_b * SM2).reshape(1, C).astype(bf),
        }
        maps.append(m)
    return maps


_CACHED_NC = None


def run(inputs, trace=False):
    global _CACHED_NC
    if _CACHED_NC is None:
        _CACHED_NC = build_program()
    maps = make_in_maps(inputs)
    res = run_bass_kernel_spmd(_CACHED_NC, maps, core_ids=list(range(B)),
                               trace=trace)
    out = np.stack([res.results[b]["out"].reshape(T, C) for b in range(B)])
    return out.astype(np.float32), res


def kernel(**inputs) -> np.ndarray:
    out, _ = run(inputs, trace=False)
    return out
